# revision 3
# baseline (speedup 1.0000x reference)
"""GNN message-passing (NORMADJ graph conv) on 8 Trainium2 NeuronCores.

Math (reference):
    d_e = pow(diags, e) with inf -> 0
    gso_1[e]  = m2 * d_e2[row[e]] * d_e3[col[e]]        edge weights
    gso_2[i]  = m1*d_e1[i] + m2*d_e2[i]*d_e3[i] + m3    self-loop weights
    out[i]    = sum_{e: col[e]==i} gso_1[e] * x[row[e]] + gso_2[i] * x[i]

Decomposition: out[i] = m2*d3[i] * sum_e (d2[row] * x[row]) + self_term[i],
with the per-edge d2[row] weight folded into the one-hot scatter matrix
(S_w[e, q] = d2[row[e]] * (col_rel[e] == q)) so x is gathered RAW (no xs
precompute roundtrip through DRAM). self_term and m2*d3 are host-computed.

Distribution: edges sharded by DESTINATION node range across 8 cores; each
core computes its output rows exactly -> no collective.

Per destination tile of 128 nodes (per core: 98 tiles):
  - edges counting-sorted by (tile, bank, dest, src); per (tile,bank) cell
    split into cap_b chunks of 128 slots (dest-sorted => each chunk's dest
    span is narrow, ~16 of 128)
  - source rows f32 gathered per (tile-group, bank) with gpsimd.dma_gather
    (int16 idx limit -> 4 banks of 25088 rows); ACT converts f32->bf16
  - one DVE is_equal + one DVE mult per tile builds the weighted one-hot
    S_w^T[e, q*KT+c] in a 2x-perf-mode-eligible layout (innermost = chunk)
  - matmuls accumulate only each chunk's dest window: psum[s_c:s_c+w_c] +=
    S_w_chunk.T @ msg_chunk, window starts quantized to {0,32,64} (PE
    base-partition constraint); a zero-matmul opens the psum tile
  - post: out = psum * (m2*d3) via ACT activation scale, + self_term (DVE)
"""

import numpy as np
import ml_dtypes

P = 128
D = 64
N_CORES = 8
N_NODES = 100000
TPC = 98                      # destination tiles per core
NPC = TPC * P                 # nodes per core (12544)
NPAD = N_CORES * NPC          # padded node count (100352)
NT = NPAD // P                # total node tiles incl. padding (784)
NB = 4                        # source banks (int16 gather index limit)
BANK = NPAD // NB             # 25088 rows per bank
T_GROUP = 4                   # dest tiles per gather/convert group

_cache = {}


def _groups():
    gs = []
    t = 0
    while t < TPC:
        gs.append((t, min(T_GROUP, TPC - t)))
        t += min(T_GROUP, TPC - t)
    return gs


def _build_program(caps, W, s_tab, w_tab, n_cores):
    import concourse.bacc as bacc
    import concourse.mybir as mybir
    from concourse.tile import TileContext

    f32 = mybir.dt.float32
    bf16 = mybir.dt.bfloat16
    i16 = mybir.dt.int16
    ACT = mybir.ActivationFunctionType

    KT = sum(caps)
    off_b = np.concatenate([[0], np.cumsum(caps)])
    groups = _groups()
    idx_cols_total = sum(tg * cb * 8 for (_, tg) in groups for cb in caps)

    nc = bacc.Bacc(
        "TRN2", target_bir_lowering=False, debug=False, num_devices=n_cores
    )

    xfull = nc.dram_tensor("xfull", [NPAD, D], f32, kind="ExternalInput")
    idx16 = nc.dram_tensor("idx16", [P, idx_cols_total], i16, kind="ExternalInput")
    colrelT = nc.dram_tensor("colrelT", [P, TPC * KT], bf16, kind="ExternalInput")
    wrelT = nc.dram_tensor("wrelT", [P, TPC * KT], bf16, kind="ExternalInput")
    iota = nc.dram_tensor("iota", [P, W * KT], bf16, kind="ExternalInput")
    zc = nc.dram_tensor("zc", [P, P], bf16, kind="ExternalInput")
    post3T = nc.dram_tensor("post3T", [P, TPC], f32, kind="ExternalInput")
    xds = nc.dram_tensor("xds", [NPC, D], f32, kind="ExternalInput")
    out_d = nc.dram_tensor("out", [NPC, D], f32, kind="ExternalOutput")

    with TileContext(nc) as tc:
        with (
            tc.tile_pool(name="const", bufs=1) as const,
            tc.tile_pool(name="idxp", bufs=3) as idxp,
            tc.tile_pool(name="msgf", bufs=5) as msgfp,
            tc.tile_pool(name="msgb", bufs=8) as msgbp,
            tc.tile_pool(name="stp", bufs=6) as stp,
            tc.tile_pool(name="outp", bufs=3) as outp,
            tc.tile_pool(name="psum", bufs=6, space="PSUM") as psum,
        ):
            iota_sb = const.tile([P, W * KT], bf16)
            nc.sync.dma_start(out=iota_sb[:], in_=iota[:])
            zc_sb = const.tile([P, P], bf16)
            nc.sync.dma_start(out=zc_sb[:], in_=zc[:])
            colT_sb = const.tile([P, TPC * KT], bf16)
            nc.sync.dma_start(out=colT_sb[:], in_=colrelT[:])
            wrelT_sb = const.tile([P, TPC * KT], bf16)
            nc.sync.dma_start(out=wrelT_sb[:], in_=wrelT[:])
            post3_sb = const.tile([P, TPC], f32)
            nc.sync.dma_start(out=post3_sb[:], in_=post3T[:])
            xds_sb = const.tile([P, TPC * D], f32)
            nc.sync.dma_start(
                out=xds_sb[:].rearrange("p (t d) -> p t d", d=D),
                in_=xds[:].rearrange("(t p) d -> p t d", p=P),
            )

            idx_off = 0
            for (t0, tg) in groups:
                gcols = sum(tg * cb * 8 for cb in caps)
                idx_sb = idxp.tile([P, gcols], i16, name="idx_sb")
                nc.sync.dma_start(
                    out=idx_sb[:], in_=idx16[:, idx_off : idx_off + gcols]
                )
                idx_off += gcols

                msgbs = []
                boff = 0
                for b in range(NB):
                    cb = caps[b]
                    nidx = tg * cb * P
                    msgf = msgfp.tile([P, tg * cb * D], f32, tag="msgf")
                    nc.gpsimd.dma_gather(
                        out_ap=msgf[:].rearrange("p (k d) -> p k d", d=D),
                        in_ap=xfull[b * BANK : (b + 1) * BANK, :],
                        idxs_ap=idx_sb[:, boff : boff + nidx // 16],
                        num_idxs=nidx,
                        num_idxs_reg=nidx,
                        elem_size=D,
                        single_packet=False,
                    )
                    boff += nidx // 16
                    msgb = msgbp.tile([P, tg * cb * D], bf16, tag="msgb")
                    nc.scalar.copy(out=msgb[:], in_=msgf[:])
                    msgbs.append(msgb)

                stage = outp.tile([P, tg * D], f32, name="stage")
                for q in range(tg):
                    t = t0 + q
                    tmp = stp.tile([P, W * KT], bf16, tag="tmp")
                    nc.vector.tensor_tensor(
                        out=tmp[:].rearrange("p (w c) -> p w c", c=KT),
                        in0=colT_sb[:, t * KT : (t + 1) * KT]
                        .rearrange("p (w c) -> p w c", w=1)
                        .to_broadcast([P, W, KT]),
                        in1=iota_sb[:].rearrange("p (w c) -> p w c", c=KT),
                        op=mybir.AluOpType.is_equal,
                    )
                    sT = stp.tile([P, W * KT], bf16, tag="sT")
                    nc.vector.tensor_tensor(
                        out=sT[:].rearrange("p (w c) -> p w c", c=KT),
                        in0=wrelT_sb[:, t * KT : (t + 1) * KT]
                        .rearrange("p (w c) -> p w c", w=1)
                        .to_broadcast([P, W, KT]),
                        in1=tmp[:].rearrange("p (w c) -> p w c", c=KT),
                        op=mybir.AluOpType.mult,
                    )
                    sT3 = sT[:].rearrange("p (w c) -> p w c", c=KT)

                    acc = psum.tile([P, D], f32, name="acc")
                    nc.tensor.matmul(
                        out=acc[:], lhsT=zc_sb[:], rhs=zc_sb[:, 0:D],
                        start=True, stop=True,
                    )
                    for b in range(NB):
                        cb = caps[b]
                        for k in range(cb):
                            c = int(off_b[b]) + k
                            s, w = s_tab[c], w_tab[c]
                            # PE constraint: out base 32 allows <=32 rows,
                            # base 64 allows <=64; split s=32 windows.
                            if s == 32 and w > 32:
                                segs = [(32, 0, 32), (64, 32, w - 32)]
                            else:
                                segs = [(s, 0, w)]
                            rhs = msgbs[b][
                                :, (q * cb + k) * D : (q * cb + k + 1) * D
                            ]
                            for (base, q0, wseg) in segs:
                                nc.tensor.matmul(
                                    out=acc[base : base + wseg, :],
                                    lhsT=sT3[:, q0 : q0 + wseg, c],
                                    rhs=rhs,
                                    start=False,
                                    stop=False,
                                    skip_group_check=True,
                                )
                    nc.scalar.activation(
                        out=stage[:, q * D : (q + 1) * D],
                        in_=acc[:],
                        func=ACT.Copy,
                        scale=post3_sb[:, t : t + 1],
                    )
                    nc.vector.tensor_add(
                        out=stage[:, q * D : (q + 1) * D],
                        in0=stage[:, q * D : (q + 1) * D],
                        in1=xds_sb[:, t * D : (t + 1) * D],
                    )
                nc.sync.dma_start(
                    out=out_d[t0 * P : (t0 + tg) * P, :].rearrange(
                        "(k p) d -> p k d", p=P
                    ),
                    in_=stage[:].rearrange("p (k d) -> p k d", d=D),
                )

    nc.compile()
    return nc


def _get_program(caps, W, s_tab, w_tab, n_cores):
    key = (tuple(caps), W, tuple(s_tab), tuple(w_tab), n_cores)
    if key not in _cache:
        _cache[key] = _build_program(
            tuple(caps), W, tuple(s_tab), tuple(w_tab), n_cores
        )
    return _cache[key]


def _pow_clean(d, e):
    with np.errstate(divide="ignore", invalid="ignore"):
        p = d.astype(np.float64) ** np.float64(e)
    p = np.where(np.isinf(p), 0.0, p)
    return p.astype(np.float32)


def plan_and_pack(x, edge_index, edge_index_id, diags, m1, m2, m3, e1, e2, e3):
    """Host-side planning (caps/windows) + index packing.

    Returns (caps, W, s_tab, w_tab, in_maps)."""
    bf16 = ml_dtypes.bfloat16
    row = np.ascontiguousarray(edge_index[0]).astype(np.int64, copy=False)
    col = np.ascontiguousarray(edge_index[1]).astype(np.int64, copy=False)
    m1f, m2f, m3f = (float(np.asarray(v).reshape(-1)[0]) for v in (m1, m2, m3))
    e1f, e2f, e3f = (float(np.asarray(v).reshape(-1)[0]) for v in (e1, e2, e3))
    diags = np.asarray(diags, np.float32)
    n = x.shape[0]

    d1 = _pow_clean(diags, e1f)
    d2 = _pow_clean(diags, e2f)
    d3 = _pow_clean(diags, e3f)

    # ---- edge ordering: (cell = tile*NB + bank) asc, dest asc, then src ----
    tile_g = col >> 7                      # global 128-node tile (0..783)
    drel = (col & 127).astype(np.int64)
    bank = row // BANK
    cell = tile_g * NB + bank              # 784*4 cells across all cores
    o1 = np.lexsort((drel, cell))
    cell_s = cell[o1]
    counts = np.bincount(cell_s, minlength=NT * NB)
    caps = np.maximum(
        np.ceil(counts.reshape(NT, NB).max(axis=0) / P).astype(int), 1
    )
    KT = int(caps.sum())
    off_b = np.concatenate([[0], np.cumsum(caps)])

    starts = np.concatenate([[0], np.cumsum(counts)[:-1]])
    pos_in_cell = np.arange(len(row)) - starts[cell_s]
    chunk = pos_in_cell >> 7               # chunk within cell
    # re-sort within each chunk by source row for gather locality
    o2 = np.lexsort((row[o1], chunk, cell_s))
    order = o1[o2]
    row_s, col_s = row[order], col[order]
    cell_f = cell_s  # cell/chunk unchanged by within-chunk permutation
    chunk_f = chunk
    pos_f = np.arange(len(row)) - starts[cell_f] - (chunk_f << 7)  # slot 0..127
    drel_f = (col_s & 127).astype(np.int64)

    # ---- window fit per (bank, k): quantized starts {0,32,64} -------------
    bank_f = cell_f % NB
    c_idx = off_b[bank_f] + chunk_f        # chunk column 0..KT-1
    dmin = np.full(KT, 128, np.int64)
    dmax = np.full(KT, -1, np.int64)
    np.minimum.at(dmin, c_idx, drel_f)
    np.maximum.at(dmax, c_idx, drel_f)
    s_tab, w_tab = [], []
    spans = []
    for c in range(KT):
        if dmax[c] < 0:
            s_tab.append(0)
            spans.append(1)
            continue
        s = min(64, 32 * (int(dmin[c]) // 32))
        s_tab.append(s)
        spans.append(int(dmax[c]) - s + 1)
    W = int(min(128, max(4, ((max(spans) + 3) // 4) * 4)))
    for c in range(KT):
        w_tab.append(min(W, 128 - s_tab[c]))
    s_arr = np.array(s_tab, np.int64)

    # ---- slot tables -------------------------------------------------------
    tcell = cell_f // NB                   # global tile of each edge
    slot_col = tcell * KT + c_idx          # column in [NT*KT) table space
    colrel_pad = np.full((NT * KT, P), -1.0, np.float32)
    wrel_pad = np.zeros((NT * KT, P), np.float32)
    idx_pad = np.zeros((NT * KT, P), np.int16)
    colrel_pad[slot_col, pos_f] = (drel_f - s_arr[c_idx]).astype(np.float32)
    wrel_pad[slot_col, pos_f] = d2[row_s]
    idx_pad[slot_col, pos_f] = (row_s - bank_f * BANK).astype(np.int16)

    # colrelT/wrelT: [core][p, t*KT + c]
    def to_core_tables(tab, dtype):
        t3 = tab.reshape(N_CORES, TPC * KT, P).transpose(0, 2, 1)
        return np.ascontiguousarray(t3).astype(dtype)

    colrelT_h = to_core_tables(colrel_pad, bf16)
    wrelT_h = to_core_tables(wrel_pad, bf16)

    # idx16: per group g, per bank b: flat seq i over (t_local, k, p);
    # value at [i % 16, base + i // 16], replicated across partition groups.
    groups = _groups()
    idx_cols_total = sum(tg * cb * 8 for (_, tg) in groups for cb in caps)
    idx16 = np.zeros((N_CORES, 16, idx_cols_total), np.int16)
    idx_pc = idx_pad.reshape(N_CORES, TPC, KT, P)
    for cc in range(N_CORES):
        base = 0
        for (t0, tg) in groups:
            for b in range(NB):
                cb = caps[b]
                nidx = tg * cb * P
                seq = idx_pc[
                    cc, t0 : t0 + tg, off_b[b] : off_b[b + 1], :
                ].reshape(nidx)
                idx16[cc, :, base : base + nidx // 16] = seq.reshape(
                    nidx // 16, 16
                ).T
                base += nidx // 16
        assert base == idx_cols_total
    idx16 = np.ascontiguousarray(
        np.broadcast_to(
            idx16[:, None, :, :], (N_CORES, 8, 16, idx_cols_total)
        ).reshape(N_CORES, P, idx_cols_total)
    )

    # ---- per-node host precomputes ----------------------------------------
    xbig = np.zeros((NPAD, D), np.float32)
    xbig[:n] = x
    post3 = np.zeros(NPAD, np.float32)
    post3[:n] = m2f * d3
    # self term via edge_index_id (reference adds positionally)
    rid = np.ascontiguousarray(edge_index_id[0]).astype(np.int64, copy=False)
    cid = np.ascontiguousarray(edge_index_id[1]).astype(np.int64, copy=False)
    gso2 = m1f * d1[rid] + m2f * (d2[rid] * d3[cid]) + m3f
    xds_full = np.zeros((NPAD, D), np.float32)
    xds_full[: len(rid)] = gso2[:, None] * x[rid]

    iota_h = np.ascontiguousarray(
        np.broadcast_to(
            np.repeat(np.arange(W, dtype=np.float32), KT)[None, :],
            (P, W * KT),
        )
    ).astype(bf16)
    zc_h = np.zeros((P, P), bf16)

    in_maps = []
    for k in range(N_CORES):
        lo, hi = k * NPC, (k + 1) * NPC
        in_maps.append(
            {
                "xfull": xbig,
                "idx16": idx16[k],
                "colrelT": colrelT_h[k],
                "wrelT": wrelT_h[k],
                "iota": iota_h,
                "zc": zc_h,
                "post3T": np.ascontiguousarray(
                    post3[lo:hi].reshape(TPC, P).T
                ),
                "xds": xds_full[lo:hi],
            }
        )
    return tuple(caps), W, tuple(s_tab), tuple(w_tab), in_maps


def kernel(x, edge_index, edge_index_id=None, diags=None, m1=None, m2=None,
           m3=None, e1=None, e2=None, e3=None, a=None, **_):
    from concourse.bass_utils import run_bass_kernel_spmd

    x = np.ascontiguousarray(np.asarray(x, dtype=np.float32))
    edge_index = np.asarray(edge_index)
    caps, W, s_tab, w_tab, in_maps = plan_and_pack(
        x, edge_index, np.asarray(edge_index_id), diags,
        m1, m2, m3, e1, e2, e3,
    )
    nc = _get_program(caps, W, s_tab, w_tab, N_CORES)
    res = run_bass_kernel_spmd(nc, in_maps, list(range(N_CORES)))
    out = np.concatenate([res.results[k]["out"] for k in range(N_CORES)], axis=0)
    return np.ascontiguousarray(out[:N_NODES])
